# revision 12
# baseline (speedup 1.0000x reference)
"""MoE (top-2 of 8 experts, SwiGLU) on 8 Trainium2 NeuronCores.

Strategy (expert-parallel, per the sharding hint):
  - Host computes the router (tiny: [2048,1024]@[1024,8]) and the top-2
    dispatch: for each expert e, the list of tokens routed to it and their
    combine weights. This IS the sharding step — each core's input shard is
    "its expert's weights + its expert's tokens".
  - Core e runs the expert MLP for its ~512 tokens:
        hT = w1[e] @ x_eT            (gate/up fused, [4096, C])
        yT = silu(hT_gate) * hT_up   ([2048, C])
        oT = (w2[e] @ yT) * combine  ([1024, C])
    GEMM1 in bf16, GEMM2 in float32r (same PE speed at moving-dim >=256),
    fp32 PSUM accumulation throughout; activations fp32.
  - Host scatter-adds the per-expert outputs back to token order (unshard).

Layouts keep tokens on the PSUM free dim everywhere so no on-device
transposes are needed; weights are pre-transposed on the host.
"""

import sys

sys.path.insert(0, "/opt/trn_rl_repo")

import numpy as np
import ml_dtypes

import concourse.bass as bass  # noqa: F401  (bass must import before tile)
import concourse.tile as tile
from concourse import bacc, mybir
from concourse.bass_utils import run_bass_kernel_spmd

T = 2048
H = 1024
INTER = 2048
E = 8
TOPK = 2
N_CORES = 8
P = 128

DT = mybir.dt.bfloat16
NP_DT = ml_dtypes.bfloat16

# GEMM2 in float32r: full fp32 operands through the PE at bf16 speed
# (1 cycle/row when the moving dim is >=256). Cuts output error from ~4e-3
# to ~3e-3; HW A/B (3 runs, 26 interleaved loop-slope pairs) shows no
# measurable speed difference vs bf16, so accuracy wins the tie.
G2_F32R = True

_PROGRAM_CACHE = {}    # c_total -> compiled Bacc program (reused across calls)

KH = H // P            # 8  k-tiles for GEMM1 (contract over H)
KI = INTER // P        # 16 k-tiles for GEMM2 (contract over INTER)
NPAIR = INTER // P     # 16 gate/up pairs
NH = H // P            # 8  output h-tiles


def _route(x, router_w):
    """Replicates the reference router in fp32 numpy.

    Returns per-expert (token_indices, combine_weights)."""
    gating = (x @ router_w.T).astype(np.float32)              # [T, E]
    m = gating.max(axis=1, keepdims=True)
    p = np.exp(gating - m, dtype=np.float32)
    probs = p / p.sum(axis=1, keepdims=True)
    order = np.argsort(-probs, axis=1, kind="stable")         # ties -> lower idx
    sel = order[:, :TOPK]                                     # [T, K]
    topw = np.take_along_axis(probs, sel, axis=1)             # [T, K]

    idxs, wts = [], []
    for e in range(E):
        m_e = sel == e                                        # [T, K]
        rows = np.nonzero(m_e.any(axis=1))[0]
        idxs.append(rows.astype(np.int64))
        wts.append(topw[m_e].astype(np.float32))              # aligned with rows
    return idxs, wts


def _chunks(c):
    """Split c tokens into near-equal chunks of <=512 (PSUM bank limit).

    Chunks are kept >=256 where possible: below that, float32r matmuls drop
    to 1/4 rate and LDWEIGHTS (~107 ns) stops hiding under the matmul."""
    n = -(-c // 512)
    base = -(-(-(-c // n)) // 4) * 4                          # ceil(c/n) to mult of 4
    sizes = []
    left = c
    for _ in range(n - 1):
        sizes.append(base)
        left -= base
    sizes.append(left)
    return [s for s in sizes if s > 0]


def _build_program(c_total, loop_n=0):
    """One SPMD program: the expert MLP for c_total (padded) tokens.

    loop_n > 0 wraps the body in an on-device For_i loop running it loop_n
    times (used only by the perf harness to measure the per-iteration slope;
    the graded path uses loop_n=0 = straight-line body)."""
    nc = bacc.Bacc("TRN2", target_bir_lowering=False, debug=False,
                   num_devices=N_CORES)
    f32 = mybir.dt.float32
    xt_d = nc.dram_tensor("xt", [H, c_total], DT, kind="ExternalInput").ap()
    w1t_d = nc.dram_tensor("w1t", [H, 2 * INTER], DT, kind="ExternalInput").ap()
    dt2 = mybir.dt.float32r if G2_F32R else DT
    w2t_d = nc.dram_tensor("w2t", [INTER, H], dt2, kind="ExternalInput").ap()
    sc_d = nc.dram_tensor("scale", [P, c_total], f32, kind="ExternalInput").ap()
    out_d = nc.dram_tensor("out", [H, c_total], f32, kind="ExternalOutput").ap()

    chunk_sizes = _chunks(c_total)

    from contextlib import ExitStack
    with tile.TileContext(nc) as tc, ExitStack() as ctx:
        wpool = ctx.enter_context(tc.tile_pool(name="weights", bufs=1))
        xpool = ctx.enter_context(tc.tile_pool(name="xt", bufs=1))
        ypool = ctx.enter_context(tc.tile_pool(name="yt",
                                               bufs=1 if G2_F32R else 2))
        apool = ctx.enter_context(tc.tile_pool(name="act", bufs=2))
        opool = ctx.enter_context(tc.tile_pool(name="ot", bufs=3))
        pgpool = ctx.enter_context(tc.tile_pool(name="psg", bufs=3, space="PSUM"))
        pupool = ctx.enter_context(tc.tile_pool(name="psu", bufs=3, space="PSUM"))
        popool = ctx.enter_context(tc.tile_pool(name="pso", bufs=2, space="PSUM"))

        # scratch tile for PE warmup matmuls; written once, outside the loop
        warm_sb = xpool.tile([P, P], DT, tag="warm")
        nc.vector.memset(warm_sb[:], 0.0)

        # ---- tiles + DMA helpers ----
        # One merged DMA per logical tensor/piece: the HWDGE prep cost is
        # per-instruction (~625 ns, serialized), so many small DMAs stall the
        # PE at startup.
        NW1P = 8
        W1PC = 2 * INTER // NW1P  # 512

        xt_t = xpool.tile([P, KH, c_total], DT, tag="xt")
        xt_view = xt_d.rearrange("(k p) c -> p k c", p=P)
        xt_sb = [xt_t[:, k, :] for k in range(KH)]
        c1 = chunk_sizes[0]

        w1_0a = wpool.tile([P, KH, 2 * P], DT, tag="w1_0a")
        w1_0a_view = w1t_d[:, :2 * P].rearrange("(k p) c -> p k c", p=P)

        # w1t column pieces (each with all 8 k-tiles), in PE consumption
        # order (gate piece p feeds pairs 4p..4p+3, paired with up piece p+4).
        # Piece 0 is split 256/256 so pairs 0-1 can start while 2-3 stream.
        w1_t = {"0a": w1_0a}
        w1_rng = {"0a": (0, 2 * P), "0b": (2 * P, W1PC)}
        w1_t["0b"] = wpool.tile([P, KH, W1PC - 2 * P], DT, tag="w1_0b",
                                name="w1_0b")
        for piece in (4, 1, 5, 2, 6, 3, 7):
            w1_t[piece] = wpool.tile([P, KH, W1PC], DT, tag=f"w1_{piece}",
                                     name=f"w1_{piece}")
            w1_rng[piece] = (piece * W1PC, (piece + 1) * W1PC)

        def load_w1_piece(key):
            lo, hi = w1_rng[key]
            nc.sync.dma_start(
                out=w1_t[key][:],
                in_=w1t_d[:, lo:hi].rearrange("(k p) c -> p k c", p=P))

        w2_t = []
        w2_sb = []
        for half in range(2):
            t = wpool.tile([P, KI // 2, H], dt2, tag=f"w2_{half}")
            w2_t.append(t)
            w2_sb.extend(t[:, k, :] for k in range(KI // 2))

        def load_w2():
            for half in range(2):
                rs = slice(half * INTER // 2, (half + 1) * INTER // 2)
                nc.sync.dma_start(
                    out=w2_t[half][:],
                    in_=w2t_d[rs, :].rearrange("(k p) c -> p k c", p=P))

        sc_sb = xpool.tile([P, c_total], f32, tag="sc")

        def load_sc():
            nc.sync.dma_start(out=sc_sb[:], in_=sc_d[:])

        # ---- prologue loads (iteration-1 inputs), gating order:
        # chunk-1 xt columns and the first w1 piece first — they unblock the
        # first matmuls.
        nc.sync.dma_start(out=xt_t[:, :KH // 2, :c1],
                          in_=xt_view[:, :KH // 2, :c1])
        nc.sync.dma_start(out=w1_0a[:, :KH // 2, :],
                          in_=w1_0a_view[:, :KH // 2, :])
        nc.sync.dma_start(out=xt_t[:, KH // 2:, :c1],
                          in_=xt_view[:, KH // 2:, :c1])
        nc.sync.dma_start(out=w1_0a[:, KH // 2:, :],
                          in_=w1_0a_view[:, KH // 2:, :])
        load_w1_piece("0b")
        if c1 < c_total:
            nc.sync.dma_start(out=xt_t[:, :, c1:], in_=xt_view[:, :, c1:])
        for piece in (4, 1, 5, 2, 6, 3, 7):
            load_w1_piece(piece)
        if not loop_n:
            # loop mode issues these at the body top instead (their WAR on
            # the previous iteration's GEMM2/scale reads resolves at the
            # back-edge, and the transfer hides under GEMM1)
            load_w2()
            load_sc()

        if loop_n:
            loop = ctx.enter_context(tc.For_i(
                0, loop_n, 1,
                staggered_reset=True,
                hint_engines=(mybir.EngineType.PE, mybir.EngineType.SP,
                              mybir.EngineType.Activation, mybir.EngineType.DVE)))

        # ---- PE warmup ----
        # ~3.5 us of dependency-free matmuls on a scratch tile: the PE HAM
        # clock-gate warms to 2.4 GHz during the initial DMA wait instead of
        # throttling the first real matmuls. The product is never read.
        # In loop mode the rotated loads keep data resident across the
        # back-edge, so no warmup burst is needed there.
        if not loop_n:
            ps_w = popool.tile([P, P], f32, tag="pso", name="ps_warm")
            # 44 matmuls: ~32 run during the HAM cold window (1.2 GHz, ~107 ns
            # each) burning it on garbage, the rest bridge until the first
            # real operands land (~3.9 us); more than ~48 delays the first
            # real matmul.
            for _ in range(44):
                nc.tensor.matmul(ps_w[:], lhsT=warm_sb[:], rhs=warm_sb[:],
                                 start=True, stop=True)

        if loop_n:
            load_w2()
            load_sc()

        def w1_slice(k, i):
            # stationary lhsT [P(h), P(inter)] for global inter tile i (0..31)
            piece, sub = divmod(i, W1PC // P)
            if piece == 0:
                if sub < 2:
                    return w1_t["0a"][:, k, P * sub:P * (sub + 1)]
                return w1_t["0b"][:, k, P * (sub - 2):P * (sub - 1)]
            return w1_t[piece][:, k, P * sub:P * (sub + 1)]

        # chunk slices (over the token free dim; PSUM caps a chunk at 512)
        csls = []
        c0 = 0
        for cn in chunk_sizes:
            csls.append((slice(c0, c0 + cn), cn))
            c0 += cn

        # ---- GEMM1 + SwiGLU: yT[i] = silu(gate_i) * up_i, [P, c_total] ----
        # Chunk loop is innermost so each w1 stationary tile is consumed
        # across the full GEMM1 span (halves the required w1 DMA bandwidth).
        # Quad structure (4 gate pairs, then their 4 ups) gives the PE ~8 us
        # of gate work from w1 piece p while up piece p+4 is still in flight.
        yt_sb = [None] * NPAIR
        for q in range(NPAIR // 4):
            quad = range(4 * q, 4 * q + 4)
            sgs = {}
            for i in quad:
                yt_sb[i] = ypool.tile([P, c_total], dt2, tag=f"yt{i}",
                                      name=f"yt{i}")
            for ci, (csl, cn) in enumerate(csls):
                for i in quad:
                    ps_g = pgpool.tile([P, cn], f32, tag="psg")
                    for k in range(KH):
                        nc.tensor.matmul(ps_g[:], lhsT=w1_slice(k, i),
                                         rhs=xt_sb[k][:, csl],
                                         start=(k == 0), stop=(k == KH - 1))
                    sg = apool.tile([P, cn], f32, tag=f"sg{i % 4}_{ci}")
                    nc.scalar.activation(sg[:], ps_g[:],
                                         mybir.ActivationFunctionType.Silu)
                    sgs[(i, ci)] = sg
            for ci, (csl, cn) in enumerate(csls):
                for i in quad:
                    ps_u = pupool.tile([P, cn], f32, tag="psu")
                    for k in range(KH):
                        nc.tensor.matmul(ps_u[:], lhsT=w1_slice(k, i + NPAIR),
                                         rhs=xt_sb[k][:, csl],
                                         start=(k == 0), stop=(k == KH - 1))
                    nc.vector.tensor_mul(yt_sb[i][:, csl], sgs[(i, ci)][:],
                                         ps_u[:])
            if loop_n:
                # rotated (software-pipelined) reloads for the NEXT
                # iteration: quad q is the last reader of gate piece q and
                # up piece q+4, so their re-DMA issues here and the data is
                # already resident when the next iteration starts.
                if q == 0:
                    nc.sync.dma_start(out=w1_0a[:], in_=w1_0a_view[:])
                    load_w1_piece("0b")
                    load_w1_piece(4)
                else:
                    load_w1_piece(q)
                    load_w1_piece(q + 4)
        if loop_n:
            # GEMM1 is the last reader of xt — reload for the next iteration
            nc.sync.dma_start(out=xt_t[:], in_=xt_view[:])

        # ---- GEMM2 + combine scale ----
        for j in range(NH):
            for csl, cn in csls:
                ps_o = popool.tile([P, cn], f32, tag="pso")
                for k in range(KI):
                    nc.tensor.matmul(ps_o[:], lhsT=w2_sb[k][:, P * j:P * (j + 1)],
                                     rhs=yt_sb[k][:, csl],
                                     start=(k == 0), stop=(k == KI - 1))
                ot = opool.tile([P, cn], f32, tag="ot")
                nc.vector.tensor_mul(ot[:], sc_sb[:, csl], ps_o[:])
                # ACT's HWDGE ring, not SP's: keeps the output stores out of
                # FIFO order with the (large) rotated weight reloads
                nc.scalar.dma_start(out=out_d[P * j:P * (j + 1), csl],
                                    in_=ot[:])

    nc.compile()
    return nc


def kernel(hidden_states, w1, w2, router_w):
    x = np.ascontiguousarray(np.asarray(hidden_states, dtype=np.float32)
                             .reshape(T, H))
    w1 = np.asarray(w1, dtype=np.float32)
    w2 = np.asarray(w2, dtype=np.float32)
    router_w = np.asarray(router_w, dtype=np.float32)

    idxs, wts = _route(x, router_w)
    c_total = max(64, -(-max(len(i) for i in idxs) // 2) * 2)

    nc = _PROGRAM_CACHE.get(c_total)
    if nc is None:
        nc = _PROGRAM_CACHE[c_total] = _build_program(c_total)

    xt_f32 = x.T  # [H, T]
    in_maps = []
    for e in range(E):
        n = len(idxs[e])
        xt = np.zeros((H, c_total), dtype=NP_DT)
        xt[:, :n] = xt_f32[:, idxs[e]].astype(NP_DT)
        sc = np.zeros((P, c_total), dtype=np.float32)
        sc[:, :n] = wts[e][None, :]
        in_maps.append({
            "xt": xt,
            "w1t": np.ascontiguousarray(w1[e].T).astype(NP_DT),
            "w2t": np.ascontiguousarray(w2[e].T).astype(
                np.float32 if G2_F32R else NP_DT),
            "scale": sc,
        })

    try:
        res = run_bass_kernel_spmd(nc, in_maps, list(range(N_CORES)))
    except Exception:
        # transient runtime hiccups (e.g. mesh desync on a fresh session)
        # usually clear on retry
        res = run_bass_kernel_spmd(nc, in_maps, list(range(N_CORES)))

    out = np.zeros((T, H), dtype=np.float32)
    for e in range(E):
        n = len(idxs[e])
        if n:
            out[idxs[e]] += res.results[e]["out"][:, :n].T
    return out.reshape(1, T, H)



# revision 13
# speedup vs baseline: 1.5671x; 1.5671x over previous
"""MoE (top-2 of 8 experts, SwiGLU) on 8 Trainium2 NeuronCores.

Strategy (expert-parallel, per the sharding hint):
  - Host computes the router (tiny: [2048,1024]@[1024,8]) and the top-2
    dispatch: for each expert e, the list of tokens routed to it and their
    combine weights. This IS the sharding step — each core's input shard is
    "its expert's weights + its expert's tokens".
  - Core e runs the expert MLP for its ~512 tokens:
        hT = w1[e] @ x_eT            (gate/up fused, [4096, C])
        yT = silu(hT_gate) * hT_up   ([2048, C])
        oT = (w2[e] @ yT) * combine  ([1024, C])
    GEMM1 in bf16, GEMM2 in float32r (same PE speed at moving-dim >=256),
    fp32 PSUM accumulation throughout; activations fp32.
  - Host scatter-adds the per-expert outputs back to token order (unshard).

Layouts keep tokens on the PSUM free dim everywhere so no on-device
transposes are needed; weights are pre-transposed on the host.
"""

import sys

sys.path.insert(0, "/opt/trn_rl_repo")

import numpy as np
import ml_dtypes

import concourse.bass as bass  # noqa: F401  (bass must import before tile)
import concourse.tile as tile
from concourse import bacc, mybir
from concourse.bass_utils import run_bass_kernel_spmd

T = 2048
H = 1024
INTER = 2048
E = 8
TOPK = 2
N_CORES = 8
P = 128

DT = mybir.dt.bfloat16
NP_DT = ml_dtypes.bfloat16

# GEMM2 in float32r: full fp32 operands through the PE at bf16 speed
# (1 cycle/row when the moving dim is >=256). Cuts output error from ~4e-3
# to ~3e-3; HW A/B (3 runs, 26 interleaved loop-slope pairs) shows no
# measurable speed difference vs bf16, so accuracy wins the tie.
G2_F32R = True

_PROGRAM_CACHE = {}    # c_total -> compiled Bacc program (reused across calls)

KH = H // P            # 8  k-tiles for GEMM1 (contract over H)
KI = INTER // P        # 16 k-tiles for GEMM2 (contract over INTER)
NPAIR = INTER // P     # 16 gate/up pairs
NH = H // P            # 8  output h-tiles


def _route(x, router_w):
    """Replicates the reference router in fp32 numpy.

    Returns per-expert (token_indices, combine_weights)."""
    gating = (x @ router_w.T).astype(np.float32)              # [T, E]
    m = gating.max(axis=1, keepdims=True)
    p = np.exp(gating - m, dtype=np.float32)
    probs = p / p.sum(axis=1, keepdims=True)
    order = np.argsort(-probs, axis=1, kind="stable")         # ties -> lower idx
    sel = order[:, :TOPK]                                     # [T, K]
    topw = np.take_along_axis(probs, sel, axis=1)             # [T, K]

    idxs, wts = [], []
    for e in range(E):
        m_e = sel == e                                        # [T, K]
        rows = np.nonzero(m_e.any(axis=1))[0]
        idxs.append(rows.astype(np.int64))
        wts.append(topw[m_e].astype(np.float32))              # aligned with rows
    return idxs, wts


def _chunks(c):
    """Split c tokens into near-equal chunks of <=512 (PSUM bank limit).

    Chunks are kept >=256 where possible: below that, float32r matmuls drop
    to 1/4 rate and LDWEIGHTS (~107 ns) stops hiding under the matmul."""
    n = -(-c // 512)
    base = -(-(-(-c // n)) // 4) * 4                          # ceil(c/n) to mult of 4
    sizes = []
    left = c
    for _ in range(n - 1):
        sizes.append(base)
        left -= base
    sizes.append(left)
    return [s for s in sizes if s > 0]


def _build_program(c_total, loop_n=0):
    """One SPMD program: the expert MLP for c_total (padded) tokens.

    loop_n > 0 wraps the body in an on-device For_i loop running it loop_n
    times (used only by the perf harness to measure the per-iteration slope;
    the graded path uses loop_n=0 = straight-line body)."""
    nc = bacc.Bacc("TRN2", target_bir_lowering=False, debug=False,
                   num_devices=N_CORES)
    f32 = mybir.dt.float32
    xt_d = nc.dram_tensor("xt", [H, c_total], DT, kind="ExternalInput").ap()
    w1t_d = nc.dram_tensor("w1t", [H, 2 * INTER], DT, kind="ExternalInput").ap()
    dt2 = mybir.dt.float32r if G2_F32R else DT
    w2t_d = nc.dram_tensor("w2t", [INTER, H], dt2, kind="ExternalInput").ap()
    sc_d = nc.dram_tensor("scale", [P, c_total], f32, kind="ExternalInput").ap()
    out_d = nc.dram_tensor("out", [H, c_total], f32, kind="ExternalOutput").ap()

    chunk_sizes = _chunks(c_total)

    from contextlib import ExitStack
    with tile.TileContext(nc) as tc, ExitStack() as ctx:
        wpool = ctx.enter_context(tc.tile_pool(name="weights", bufs=1))
        xpool = ctx.enter_context(tc.tile_pool(name="xt", bufs=1))
        ypool = ctx.enter_context(tc.tile_pool(name="yt",
                                               bufs=1 if G2_F32R else 2))
        apool = ctx.enter_context(tc.tile_pool(name="act", bufs=2))
        opool = ctx.enter_context(tc.tile_pool(name="ot", bufs=3))
        pgpool = ctx.enter_context(tc.tile_pool(name="psg", bufs=2, space="PSUM"))
        pupool = ctx.enter_context(tc.tile_pool(name="psu", bufs=3, space="PSUM"))
        popool = ctx.enter_context(tc.tile_pool(name="pso", bufs=3, space="PSUM"))

        # scratch tile for PE warmup matmuls; written once, outside the loop
        warm_sb = xpool.tile([P, P], DT, tag="warm")
        nc.vector.memset(warm_sb[:], 0.0)

        # ---- tiles + DMA helpers ----
        # One merged DMA per logical tensor/piece: the HWDGE prep cost is
        # per-instruction (~625 ns, serialized), so many small DMAs stall the
        # PE at startup.
        NW1P = 8
        W1PC = 2 * INTER // NW1P  # 512

        xt_t = xpool.tile([P, KH, c_total], DT, tag="xt")
        xt_view = xt_d.rearrange("(k p) c -> p k c", p=P)
        xt_sb = [xt_t[:, k, :] for k in range(KH)]
        c1 = chunk_sizes[0]

        w1_0a = wpool.tile([P, KH, 2 * P], DT, tag="w1_0a")
        w1_0a_view = w1t_d[:, :2 * P].rearrange("(k p) c -> p k c", p=P)

        # w1t column pieces (each with all 8 k-tiles), in PE consumption
        # order (gate piece p feeds pairs 4p..4p+3, paired with up piece p+4).
        # Piece 0 is split 256/256 so pairs 0-1 can start while 2-3 stream.
        w1_t = {"0a": w1_0a}
        w1_rng = {"0a": (0, 2 * P), "0b": (2 * P, W1PC)}
        w1_t["0b"] = wpool.tile([P, KH, W1PC - 2 * P], DT, tag="w1_0b",
                                name="w1_0b")
        for piece in (4, 1, 5, 2, 6, 3, 7):
            w1_t[piece] = wpool.tile([P, KH, W1PC], DT, tag=f"w1_{piece}",
                                     name=f"w1_{piece}")
            w1_rng[piece] = (piece * W1PC, (piece + 1) * W1PC)

        def load_w1_piece(key):
            lo, hi = w1_rng[key]
            nc.sync.dma_start(
                out=w1_t[key][:],
                in_=w1t_d[:, lo:hi].rearrange("(k p) c -> p k c", p=P))

        w2_t = []
        w2_sb = []
        for half in range(2):
            t = wpool.tile([P, KI // 2, H], dt2, tag=f"w2_{half}")
            w2_t.append(t)
            w2_sb.extend(t[:, k, :] for k in range(KI // 2))

        def load_w2():
            for half in range(2):
                rs = slice(half * INTER // 2, (half + 1) * INTER // 2)
                nc.sync.dma_start(
                    out=w2_t[half][:],
                    in_=w2t_d[rs, :].rearrange("(k p) c -> p k c", p=P))

        sc_sb = xpool.tile([P, c_total], f32, tag="sc")

        def load_sc():
            nc.sync.dma_start(out=sc_sb[:], in_=sc_d[:])

        # ---- prologue loads (iteration-1 inputs), gating order:
        # chunk-1 xt columns and the first w1 piece first — they unblock the
        # first matmuls.
        nc.sync.dma_start(out=xt_t[:, :KH // 2, :c1],
                          in_=xt_view[:, :KH // 2, :c1])
        nc.sync.dma_start(out=w1_0a[:, :KH // 2, :],
                          in_=w1_0a_view[:, :KH // 2, :])
        nc.sync.dma_start(out=xt_t[:, KH // 2:, :c1],
                          in_=xt_view[:, KH // 2:, :c1])
        nc.sync.dma_start(out=w1_0a[:, KH // 2:, :],
                          in_=w1_0a_view[:, KH // 2:, :])
        load_w1_piece("0b")
        if c1 < c_total:
            nc.sync.dma_start(out=xt_t[:, :, c1:], in_=xt_view[:, :, c1:])
        for piece in (4, 1, 5, 2, 6, 3, 7):
            load_w1_piece(piece)
        if not loop_n:
            # loop mode issues these at the body top instead (their WAR on
            # the previous iteration's GEMM2/scale reads resolves at the
            # back-edge, and the transfer hides under GEMM1)
            load_w2()
            load_sc()

        if loop_n:
            loop = ctx.enter_context(tc.For_i(
                0, loop_n, 1,
                staggered_reset=True,
                hint_engines=(mybir.EngineType.PE, mybir.EngineType.SP,
                              mybir.EngineType.Activation, mybir.EngineType.DVE)))

        # ---- PE warmup ----
        # ~3.5 us of dependency-free matmuls on a scratch tile: the PE HAM
        # clock-gate warms to 2.4 GHz during the initial DMA wait instead of
        # throttling the first real matmuls. The product is never read.
        # In loop mode the rotated loads keep data resident across the
        # back-edge, so no warmup burst is needed there.
        if not loop_n:
            ps_w = popool.tile([P, P], f32, tag="pso", name="ps_warm")
            # 44 matmuls: ~32 run during the HAM cold window (1.2 GHz, ~107 ns
            # each) burning it on garbage, the rest bridge until the first
            # real operands land (~3.9 us); more than ~48 delays the first
            # real matmul.
            for _ in range(44):
                nc.tensor.matmul(ps_w[:], lhsT=warm_sb[:], rhs=warm_sb[:],
                                 start=True, stop=True)

        if loop_n:
            load_w2()
            load_sc()

        def w1_slice(k, i):
            # stationary lhsT [P(h), P(inter)] for global inter tile i (0..31)
            piece, sub = divmod(i, W1PC // P)
            if piece == 0:
                if sub < 2:
                    return w1_t["0a"][:, k, P * sub:P * (sub + 1)]
                return w1_t["0b"][:, k, P * (sub - 2):P * (sub - 1)]
            return w1_t[piece][:, k, P * sub:P * (sub + 1)]

        # chunk slices (over the token free dim; PSUM caps a chunk at 512)
        csls = []
        c0 = 0
        for cn in chunk_sizes:
            csls.append((slice(c0, c0 + cn), cn))
            c0 += cn

        # ---- GEMM1 + SwiGLU: yT[i] = silu(gate_i) * up_i, [P, c_total] ----
        # Chunk loop is innermost so each w1 stationary tile is consumed
        # across the full GEMM1 span (halves the required w1 DMA bandwidth).
        # Quad structure (4 gate pairs, then their 4 ups) gives the PE ~8 us
        # of gate work from w1 piece p while up piece p+4 is still in flight.
        yt_sb = [None] * NPAIR
        for q in range(NPAIR // 4):
            quad = range(4 * q, 4 * q + 4)
            sgs = {}
            for i in quad:
                yt_sb[i] = ypool.tile([P, c_total], dt2, tag=f"yt{i}",
                                      name=f"yt{i}")
            for ci, (csl, cn) in enumerate(csls):
                for i in quad:
                    ps_g = pgpool.tile([P, cn], f32, tag="psg")
                    for k in range(KH):
                        nc.tensor.matmul(ps_g[:], lhsT=w1_slice(k, i),
                                         rhs=xt_sb[k][:, csl],
                                         start=(k == 0), stop=(k == KH - 1))
                    sg = apool.tile([P, cn], f32, tag=f"sg{i % 4}_{ci}")
                    nc.scalar.activation(sg[:], ps_g[:],
                                         mybir.ActivationFunctionType.Silu)
                    sgs[(i, ci)] = sg
            for ci, (csl, cn) in enumerate(csls):
                for i in quad:
                    ps_u = pupool.tile([P, cn], f32, tag="psu")
                    for k in range(KH):
                        nc.tensor.matmul(ps_u[:], lhsT=w1_slice(k, i + NPAIR),
                                         rhs=xt_sb[k][:, csl],
                                         start=(k == 0), stop=(k == KH - 1))
                    nc.vector.tensor_mul(yt_sb[i][:, csl], sgs[(i, ci)][:],
                                         ps_u[:])
            if loop_n:
                # rotated (software-pipelined) reloads for the NEXT
                # iteration: quad q is the last reader of gate piece q and
                # up piece q+4, so their re-DMA issues here and the data is
                # already resident when the next iteration starts.
                if q == 0:
                    nc.sync.dma_start(out=w1_0a[:], in_=w1_0a_view[:])
                    load_w1_piece("0b")
                    load_w1_piece(4)
                else:
                    load_w1_piece(q)
                    load_w1_piece(q + 4)
        if loop_n:
            # GEMM1 is the last reader of xt — reload for the next iteration
            nc.sync.dma_start(out=xt_t[:], in_=xt_view[:])

        # ---- GEMM2 + combine scale ----
        for j in range(NH):
            for csl, cn in csls:
                ps_o = popool.tile([P, cn], f32, tag="pso")
                for k in range(KI):
                    nc.tensor.matmul(ps_o[:], lhsT=w2_sb[k][:, P * j:P * (j + 1)],
                                     rhs=yt_sb[k][:, csl],
                                     start=(k == 0), stop=(k == KI - 1))
                ot = opool.tile([P, cn], f32, tag="ot")
                nc.vector.tensor_mul(ot[:], sc_sb[:, csl], ps_o[:])
                # ACT's HWDGE ring, not SP's: keeps the output stores out of
                # FIFO order with the (large) rotated weight reloads
                nc.scalar.dma_start(out=out_d[P * j:P * (j + 1), csl],
                                    in_=ot[:])

    nc.compile()
    return nc


def kernel(hidden_states, w1, w2, router_w):
    x = np.ascontiguousarray(np.asarray(hidden_states, dtype=np.float32)
                             .reshape(T, H))
    w1 = np.asarray(w1, dtype=np.float32)
    w2 = np.asarray(w2, dtype=np.float32)
    router_w = np.asarray(router_w, dtype=np.float32)

    idxs, wts = _route(x, router_w)
    c_total = max(64, -(-max(len(i) for i in idxs) // 2) * 2)

    nc = _PROGRAM_CACHE.get(c_total)
    if nc is None:
        nc = _PROGRAM_CACHE[c_total] = _build_program(c_total)

    xt_f32 = x.T  # [H, T]
    in_maps = []
    for e in range(E):
        n = len(idxs[e])
        xt = np.zeros((H, c_total), dtype=NP_DT)
        xt[:, :n] = xt_f32[:, idxs[e]].astype(NP_DT)
        sc = np.zeros((P, c_total), dtype=np.float32)
        sc[:, :n] = wts[e][None, :]
        in_maps.append({
            "xt": xt,
            "w1t": np.ascontiguousarray(w1[e].T).astype(NP_DT),
            "w2t": np.ascontiguousarray(w2[e].T).astype(
                np.float32 if G2_F32R else NP_DT),
            "scale": sc,
        })

    try:
        res = run_bass_kernel_spmd(nc, in_maps, list(range(N_CORES)))
    except Exception:
        # transient runtime hiccups (e.g. mesh desync on a fresh session)
        # usually clear on retry
        res = run_bass_kernel_spmd(nc, in_maps, list(range(N_CORES)))

    out = np.zeros((T, H), dtype=np.float32)
    for e in range(E):
        n = len(idxs[e])
        if n:
            out[idxs[e]] += res.results[e]["out"][:, :n].T
    return out.reshape(1, T, H)

